# revision 22
# baseline (speedup 1.0000x reference)
"""Trainium2 Bass kernel for nn_BlockDiagonalLinear_text (hyperbolic block-diag linear).

Math: the reference's per-row operations reduce to
  out = alpha_row * y   with  y = x @ blockdiag(W_1..W_16).T
where alpha_row is a chain of tanh/artanh scalars of ||x_row|| and
||y_row|| (the expmap0 scale cancels; validated numerically at 1.6e-4).

Sharding: data-parallel over rows; 8192 rows -> 8 cores x 1024 rows,
weights replicated (bf16).

Layout: x is transposed and cast to bf16 on the HOST, so the device
receives xT [D, rows] k-major — the exact stationary-operand layout the
PE needs (no on-chip transposes) at half the fp32 input bytes. Output
is written bf16 and upcast on the host, halving the output DMA.

Per-core schedule: 16 block-DMAs of xT stream on the scalar-engine
HWDGE ring while w streams on the sync ring. Row-tiles run in waves
[4,2,1,1]: wave 0 (4 tiles) is paced by the input stream, later waves
run from SBUF, and the two single-tile waves keep the final
drain+norm+scale+store tail short. Per (chunk, row-tile): y block
matmul (N=256) + Gram matmul (N=128, diag = ||x||^2) sharing the
stationary operand. PSUM y groups ([128,512] = 2 blocks) drain to bf16
via ACT (2 of 8 groups) and DVE (the rest); ACT Square+accum over
drained y gives ||y||^2; a per-wave Ln/Exp scalar chain produces
alpha; DVE scales y in place (bf16 4x mode) and the result streams out
over SWDGE.
"""
import sys
import numpy as np

for _p in ("/opt/trn_rl_repo", "/root/.axon_site/_ro/trn_rl_repo"):
    if _p not in sys.path:
        sys.path.append(_p)

import ml_dtypes
import concourse.bass as bass
import concourse.bacc as bacc
import concourse.mybir as mybir
from concourse import tile
from concourse.bass_utils import run_bass_kernel_spmd
from concourse.hw_specs import get_activation_tables

R, BS = 16, 256           # 16 diagonal blocks of 256x256
D = R * BS                # 4096
P = 128                   # partitions
N_CORES = 8
ROWS_TOTAL = 4 * 2048     # 8192
ROWS_CORE = ROWS_TOTAL // N_CORES   # 1024
NT = ROWS_CORE // P       # 8 row-tiles of 128 rows per core
NC = D // P               # 32 k-chunks of 128
WCOLS = 2 * R * BS        # 8192 weight columns

# wave 0 hides under the input stream; the trailing 1-tile waves keep
# the end-of-kernel drain/scale/store tail short
WAVES = [(0, 1, 2, 3), (4, 5), (6,), (7,)]
PY_BUFS = {0: 2, 1: 2, 2: 1, 3: 1}   # PSUM: 12KB y + 4KB gram per part

f32 = mybir.dt.float32
bf16 = mybir.dt.bfloat16
AF = mybir.ActivationFunctionType
OP = mybir.AluOpType

CLIP_Z = float(np.float32(1.0) - np.float32(1e-5))          # 0.99999
MAXNORM = float(np.float32(1.0 - 1e-3) / np.float32(0.1))   # 9.99
# artanh(min(tanh(t), c)) == min(t, artanh(c)) -- the clamps collapse to
# min-with-constant, removing both tanh+artanh evaluations from the chain
ATH_CLIPZ = float(np.arctanh(np.float64(CLIP_Z)))           # 6.1030
ATH_MAXN = float(np.arctanh(np.float64(np.float32(0.1) * np.float32(MAXNORM))))


def build_nc():
    nc = bacc.Bacc()
    xt_d = nc.declare_dram_parameter("xt", [D, ROWS_CORE], bf16, isOutput=False)
    w_d = nc.declare_dram_parameter("w", [P, WCOLS], bf16, isOutput=False)
    m_d = nc.declare_dram_parameter("idm", [P, P], f32, isOutput=False)
    out_d = nc.declare_dram_parameter("out", [ROWS_CORE, D], bf16, isOutput=True)

    tabs = list(get_activation_tables(nc.m.arch).items())
    nle_id = next(i for i, (n, _) in enumerate(tabs)
                  if n == "natural_log_exp_and_others")

    with tile.TileContext(nc) as tc:
        with (
            tc.tile_pool(name="wpool", bufs=1) as wpool,
            tc.tile_pool(name="xpool", bufs=1) as xpool,
            tc.tile_pool(name="ypool", bufs=1) as ypool,
            tc.tile_pool(name="sqpool", bufs=2) as sqpool,
            tc.tile_pool(name="stats", bufs=3) as stats,
            tc.tile_pool(name="psy", bufs=2, space="PSUM") as psy,
            tc.tile_pool(name="psg", bufs=2, space="PSUM") as psg,
        ):
            V = nc.vector

            # x streams on the sync-engine HWDGE ring; the small w + idm
            # loads go on the scalar ring, which drains before ACT's
            # first compute is needed — two descriptor rings in parallel
            xt_sb = xpool.tile([P, NC * ROWS_CORE], bf16, name="xt_sb")
            for b in range(R):
                src = xt_d[b * 2 * P:(b + 1) * 2 * P, :].rearrange(
                    "(c p) r -> p c r", p=P)
                eng = nc.sync if b % 2 == 0 else nc.gpsimd
                eng.dma_start(
                    out=xt_sb[:, b * 2 * ROWS_CORE:(b + 1) * 2 * ROWS_CORE],
                    in_=src)
            w_sb = wpool.tile([P, WCOLS], bf16, name="w_sb")
            for b in range(8):
                nc.scalar.dma_start(
                    out=w_sb[:, b * 1024:(b + 1) * 1024],
                    in_=w_d[:, b * 1024:(b + 1) * 1024])
            idm_sb = wpool.tile([P, P], f32, name="idm_sb")
            nc.scalar.dma_start(out=idm_sb[:], in_=m_d[:])

            # ACT: preload the one table set with ln+exp+copy so the
            # auto-inserted per-function loads all become no-ops.
            nc.scalar.add_instruction(mybir.InstLoadActFuncSet(
                name=nc.get_next_instruction_name(),
                act_func_set_id=nle_id, ins=[], outs=[]))

            def xs(kc, rt):
                # lhsT slice: [k=128, rows 128] of chunk kc, row-tile rt
                base = kc * ROWS_CORE + rt * P
                return xt_sb[:, base:base + P]

            def st(shape, tag):
                return stats.tile(shape, f32, tag=tag, name=tag)

            y_sbs = [ypool.tile([P, D], bf16, name=f"y_{rt}") for rt in range(NT)]


            def emit_chain(qq, c, wave, split_out):
                # qq: [P, 2c] = [qx cols | qy cols]
                lnq = st([P, 2 * c], "lnq")
                nc.scalar.activation(lnq[:], qq[:], AF.Ln)
                U = st([P, 2 * c], "U")   # [u | y_n] = sqrt via exp(.5 ln q)
                nc.scalar.activation(U[:], lnq[:], AF.Exp, scale=0.5)
                t1 = st([P, c], "t1")     # 0.1 * max(u, 1e-5)
                V.tensor_scalar(out=t1[:], in0=U[:, 0:c], scalar1=1e-5,
                                scalar2=0.1, op0=OP.max, op1=OP.mult)
                r1 = st([P, c], "r1")
                V.reciprocal(r1[:], t1[:])
                d_ = st([P, c], "d_")     # 2*artanh(min(tanh(t1), CLIP_Z))
                V.tensor_scalar(out=d_[:], in0=t1[:], scalar1=ATH_CLIPZ,
                                scalar2=2.0, op0=OP.min, op1=OP.mult)
                yns = st([P, c], "yns")
                V.tensor_scalar_max(yns[:], U[:, c:2 * c], 1e-20)
                w1 = st([P, c], "w1")
                V.tensor_mul(w1[:], U[:, c:2 * c], r1[:])
                w2 = st([P, c], "w2")
                V.tensor_mul(w2[:], w1[:], d_[:])
                argt = st([P, c], "argt")
                V.tensor_scalar(out=argt[:], in0=w2[:], scalar1=0.05,
                                scalar2=15.0, op0=OP.mult, op1=OP.min)
                # tanh(argt)/max(10*tanh(argt),1e-5) == min(1e5*argt, 0.1)
                # exactly in fp32, so the second tanh cancels out of alpha
                cf = st([P, c], "cf")
                V.tensor_scalar(out=cf[:], in0=argt[:], scalar1=1e5,
                                scalar2=0.1, op0=OP.mult, op1=OP.min)
                ryn = st([P, c], "ryn")
                V.reciprocal(ryn[:], yns[:])
                db = st([P, c], "db")
                V.tensor_scalar(out=db[:], in0=argt[:], scalar1=ATH_MAXN,
                                scalar2=2.0, op0=OP.min, op1=OP.mult)
                a1 = st([P, c], "a1")
                V.tensor_mul(a1[:], ryn[:], db[:])
                a2 = st([P, c], "a2")
                V.tensor_mul(a2[:], a1[:], cf[:])
                mask = st([P, c], "mask")
                V.tensor_scalar(out=mask[:], in0=qq[:, c:2 * c], scalar1=0.0,
                                scalar2=None, op0=OP.is_gt)
                alm = st([P, c], "alm")
                V.tensor_mul(alm[:], a2[:], mask[:])
                # scale in place (bf16 4x mode) + DMA out; factor 50 folds
                # the logmap 10/nrm and the artanh halves. Single-tile
                # waves scale+store in halves so the two DMAs overlap.
                for i, rt in enumerate(wave):
                    yt = y_sbs[rt]
                    nh = split_out
                    for h in range(nh):
                        sl = slice(h * (D // nh), (h + 1) * (D // nh))
                        V.tensor_scalar(out=yt[:, sl], in0=yt[:, sl],
                                        scalar1=alm[:, i:i + 1], scalar2=50.0,
                                        op0=OP.mult, op1=OP.mult)
                        nc.gpsimd.dma_start(
                            out=out_d[rt * P:(rt + 1) * P, sl],
                            in_=yt[:, sl])

            for wi, wave in enumerate(WAVES):
                cw = len(wave)
                # one shared PSUM tile for the wave's Gram accumulators:
                # per-rt column slices would be concurrent accumulation
                # groups in one 2KB zero region, so zero it explicitly and
                # accumulate with start=False throughout
                gram = psg.tile([P, cw * P], f32, tag="gram", name="gram")
                V.memset(gram[:], 0.0)
                qp = st([P, cw * 4], "qp")
                # early waves batch the qy squares big; late waves keep
                # them fine-grained so alpha (and the store) comes sooner
                sq_span = 4 if cw > 1 else 2
                for g in range(8):          # 512-col groups: blocks 2g, 2g+1
                    for i, rt in enumerate(wave):
                        py = psy.tile([P, 512], f32, tag=f"py{i}",
                                      name=f"py{i}", bufs=PY_BUFS[i])
                        for c in range(4):  # chunks 4g .. 4g+3
                            kc = 4 * g + c
                            lhs = xs(kc, rt)
                            nc.tensor.matmul(
                                py[:, (c // 2) * BS:(c // 2 + 1) * BS],
                                lhs, w_sb[:, kc * BS:(kc + 1) * BS],
                                start=(c % 2 == 0), stop=(c % 2 == 1),
                            )
                            nc.tensor.matmul(
                                gram[:, i * P:(i + 1) * P], lhs, lhs,
                                start=False, stop=False,
                                skip_group_check=True,
                            )
                        # drain PSUM -> bf16 y; ACT takes 2 of 8 groups,
                        # DVE the rest (ACT's budget goes to the squares)
                        y_sl = y_sbs[rt][:, g * 512:(g + 1) * 512]
                        if g in (2, 6):
                            nc.scalar.activation(y_sl, py[:], AF.Copy)
                        else:
                            V.tensor_copy(y_sl, py[:])
                        if g % sq_span == sq_span - 1:
                            # qy partial over the last sq_span groups
                            sq = sqpool.tile([P, 512 * sq_span], bf16,
                                             tag="sq", name="sq")
                            nc.scalar.activation(
                                sq[:],
                                y_sbs[rt][:, (g + 1 - sq_span) * 512:
                                          (g + 1) * 512],
                                AF.Square,
                                accum_out=qp[:, i * 4 + g // sq_span:
                                             i * 4 + g // sq_span + 1])
                # wave end: qx from gram diagonals, qy from qp sums
                nparts = 8 // sq_span
                qq = st([P, 2 * cw], "qq")
                for i, rt in enumerate(wave):
                    gsc = sqpool.tile([P, P], f32, tag="gsc", name="gsc")
                    V.tensor_mul(gsc[:], gram[:, i * P:(i + 1) * P], idm_sb[:])
                    V.reduce_sum(qq[:, i:i + 1], gsc[:],
                                 axis=mybir.AxisListType.X)
                    V.reduce_sum(qq[:, cw + i:cw + i + 1],
                                 qp[:, i * 4:i * 4 + nparts],
                                 axis=mybir.AxisListType.X)
                emit_chain(qq, cw, wave,
                           split_out=(2 if cw == 1 else 1))
    nc.finalize()
    return nc


_NC = None


def _get_nc():
    global _NC
    if _NC is None:
        _NC = build_nc()
    return _NC


def _prep_weights(weights: np.ndarray) -> np.ndarray:
    # w_sb[p, (2r+c)*256+j] = W[r, j, k=c*128+p]; bf16.
    wt = (weights.astype(np.float32).transpose(0, 2, 1)      # [r, k, j]
          .reshape(R, 2, P, BS).transpose(2, 0, 1, 3)        # [p, r, c, j]
          .reshape(P, WCOLS))
    return np.ascontiguousarray(wt).astype(ml_dtypes.bfloat16)


def _in_maps(x: np.ndarray, weights: np.ndarray) -> list:
    xf = np.ascontiguousarray(x, dtype=np.float32).reshape(
        N_CORES, ROWS_CORE, D)
    # host-side transpose to k-major + bf16 cast: [core, D, rows]
    xts = xf.transpose(0, 2, 1).astype(ml_dtypes.bfloat16)
    wid = _prep_weights(np.asarray(weights))
    idm = np.eye(P, dtype=np.float32)
    return [
        {"xt": np.ascontiguousarray(xts[i]), "w": wid, "idm": idm}
        for i in range(N_CORES)
    ]


def kernel(x: np.ndarray, weights: np.ndarray) -> np.ndarray:
    nc = _get_nc()
    in_maps = _in_maps(x, weights)
    res = run_bass_kernel_spmd(nc, in_maps, list(range(N_CORES)))
    out = np.concatenate([res.results[i]["out"] for i in range(N_CORES)],
                         axis=0)
    return out.reshape(x.shape).astype(np.float32)


if __name__ == "__main__":
    xs = np.random.randn(4, 2048, D).astype(np.float32)
    ws = (np.broadcast_to(np.eye(BS, dtype=np.float32), (R, BS, BS))
          + 0.02 * np.random.randn(R, BS, BS).astype(np.float32))
    o = kernel(xs, ws)
    print("kernel ran, out shape", o.shape, o.dtype)


# revision 23
# speedup vs baseline: 1.1301x; 1.1301x over previous
"""Trainium2 Bass kernel for nn_BlockDiagonalLinear_text (hyperbolic block-diag linear).

Math: the reference's per-row operations reduce to
  out = alpha_row * y   with  y = x @ blockdiag(W_1..W_16).T
where alpha_row is a chain of tanh/artanh scalars of ||x_row|| and
||y_row|| (the expmap0 scale cancels; validated numerically at 1.6e-4).

Sharding: data-parallel over rows; 8192 rows -> 8 cores x 1024 rows,
weights replicated (bf16).

Layout: x is transposed and cast to bf16 on the HOST, so the device
receives xT [D, rows] k-major — the exact stationary-operand layout the
PE needs (no on-chip transposes) at half the fp32 input bytes. Output
is written bf16 and upcast on the host, halving the output DMA.

Per-core schedule: 16 block-DMAs of xT stream on the scalar-engine
HWDGE ring while w streams on the sync ring. Row-tiles run in waves
[4,2,1,1]: wave 0 (4 tiles) is paced by the input stream, later waves
run from SBUF, and the two single-tile waves keep the final
drain+norm+scale+store tail short. Per (chunk, row-tile): y block
matmul (N=256) + Gram matmul (N=128, diag = ||x||^2) sharing the
stationary operand. PSUM y groups ([128,512] = 2 blocks) drain to bf16
via ACT (2 of 8 groups) and DVE (the rest); ACT Square+accum over
drained y gives ||y||^2; a per-wave Ln/Exp scalar chain produces
alpha; DVE scales y in place (bf16 4x mode) and the result streams out
over SWDGE.
"""
import sys
import numpy as np

for _p in ("/opt/trn_rl_repo", "/root/.axon_site/_ro/trn_rl_repo"):
    if _p not in sys.path:
        sys.path.append(_p)

import ml_dtypes
import concourse.bass as bass
import concourse.bacc as bacc
import concourse.mybir as mybir
from concourse import tile
from concourse.bass_utils import run_bass_kernel_spmd
from concourse.hw_specs import get_activation_tables

R, BS = 16, 256           # 16 diagonal blocks of 256x256
D = R * BS                # 4096
P = 128                   # partitions
N_CORES = 8
ROWS_TOTAL = 4 * 2048     # 8192
ROWS_CORE = ROWS_TOTAL // N_CORES   # 1024
NT = ROWS_CORE // P       # 8 row-tiles of 128 rows per core
NC = D // P               # 32 k-chunks of 128
WCOLS = 2 * R * BS        # 8192 weight columns

# wave 0 hides under the input stream; the trailing 1-tile waves keep
# the end-of-kernel drain/scale/store tail short
WAVES = [(0, 1, 2, 3), (4, 5), (6,), (7,)]
PY_BUFS = {0: 2, 1: 2, 2: 1, 3: 1}   # PSUM: 12KB y + 4KB gram per part

f32 = mybir.dt.float32
bf16 = mybir.dt.bfloat16
AF = mybir.ActivationFunctionType
OP = mybir.AluOpType

CLIP_Z = float(np.float32(1.0) - np.float32(1e-5))          # 0.99999
MAXNORM = float(np.float32(1.0 - 1e-3) / np.float32(0.1))   # 9.99
# artanh(min(tanh(t), c)) == min(t, artanh(c)) -- the clamps collapse to
# min-with-constant, removing both tanh+artanh evaluations from the chain
ATH_CLIPZ = float(np.arctanh(np.float64(CLIP_Z)))           # 6.1030
ATH_MAXN = float(np.arctanh(np.float64(np.float32(0.1) * np.float32(MAXNORM))))


def build_nc():
    nc = bacc.Bacc()
    xt_d = nc.declare_dram_parameter("xt", [D, ROWS_CORE], bf16, isOutput=False)
    w_d = nc.declare_dram_parameter("w", [P, WCOLS], bf16, isOutput=False)
    m_d = nc.declare_dram_parameter("idm", [P, P], f32, isOutput=False)
    out_d = nc.declare_dram_parameter("out", [ROWS_CORE, D], bf16, isOutput=True)

    tabs = list(get_activation_tables(nc.m.arch).items())
    nle_id = next(i for i, (n, _) in enumerate(tabs)
                  if n == "natural_log_exp_and_others")

    with tile.TileContext(nc) as tc:
        with (
            tc.tile_pool(name="wpool", bufs=1) as wpool,
            tc.tile_pool(name="xpool", bufs=1) as xpool,
            tc.tile_pool(name="ypool", bufs=1) as ypool,
            tc.tile_pool(name="sqpool", bufs=2) as sqpool,
            tc.tile_pool(name="stats", bufs=3) as stats,
            tc.tile_pool(name="psy", bufs=2, space="PSUM") as psy,
            tc.tile_pool(name="psg", bufs=2, space="PSUM") as psg,
        ):
            V = nc.vector

            # x streams on the sync-engine HWDGE ring; the small w + idm
            # loads go on the scalar ring, which drains before ACT's
            # first compute is needed — two descriptor rings in parallel
            xt_sb = xpool.tile([P, NC * ROWS_CORE], bf16, name="xt_sb")
            for b in range(R):
                src = xt_d[b * 2 * P:(b + 1) * 2 * P, :].rearrange(
                    "(c p) r -> p c r", p=P)
                nc.sync.dma_start(
                    out=xt_sb[:, b * 2 * ROWS_CORE:(b + 1) * 2 * ROWS_CORE],
                    in_=src)
            w_sb = wpool.tile([P, WCOLS], bf16, name="w_sb")
            for b in range(8):
                nc.scalar.dma_start(
                    out=w_sb[:, b * 1024:(b + 1) * 1024],
                    in_=w_d[:, b * 1024:(b + 1) * 1024])
            idm_sb = wpool.tile([P, P], f32, name="idm_sb")
            nc.scalar.dma_start(out=idm_sb[:], in_=m_d[:])

            # ACT: preload the one table set with ln+exp+copy so the
            # auto-inserted per-function loads all become no-ops.
            nc.scalar.add_instruction(mybir.InstLoadActFuncSet(
                name=nc.get_next_instruction_name(),
                act_func_set_id=nle_id, ins=[], outs=[]))

            def xs(kc, rt):
                # lhsT slice: [k=128, rows 128] of chunk kc, row-tile rt
                base = kc * ROWS_CORE + rt * P
                return xt_sb[:, base:base + P]

            def st(shape, tag):
                return stats.tile(shape, f32, tag=tag, name=tag)

            y_sbs = [ypool.tile([P, D], bf16, name=f"y_{rt}") for rt in range(NT)]


            def emit_chain(qq, c, wave, split_out):
                # qq: [P, 2c] = [qx cols | qy cols]
                lnq = st([P, 2 * c], "lnq")
                nc.scalar.activation(lnq[:], qq[:], AF.Ln)
                U = st([P, 2 * c], "U")   # [u | y_n] = sqrt via exp(.5 ln q)
                nc.scalar.activation(U[:], lnq[:], AF.Exp, scale=0.5)
                t1 = st([P, c], "t1")     # 0.1 * max(u, 1e-5)
                V.tensor_scalar(out=t1[:], in0=U[:, 0:c], scalar1=1e-5,
                                scalar2=0.1, op0=OP.max, op1=OP.mult)
                r1 = st([P, c], "r1")
                V.reciprocal(r1[:], t1[:])
                d_ = st([P, c], "d_")     # 2*artanh(min(tanh(t1), CLIP_Z))
                V.tensor_scalar(out=d_[:], in0=t1[:], scalar1=ATH_CLIPZ,
                                scalar2=2.0, op0=OP.min, op1=OP.mult)
                yns = st([P, c], "yns")
                V.tensor_scalar_max(yns[:], U[:, c:2 * c], 1e-20)
                w1 = st([P, c], "w1")
                V.tensor_mul(w1[:], U[:, c:2 * c], r1[:])
                w2 = st([P, c], "w2")
                V.tensor_mul(w2[:], w1[:], d_[:])
                argt = st([P, c], "argt")
                V.tensor_scalar(out=argt[:], in0=w2[:], scalar1=0.05,
                                scalar2=15.0, op0=OP.mult, op1=OP.min)
                # tanh(argt)/max(10*tanh(argt),1e-5) == min(1e5*argt, 0.1)
                # exactly in fp32, so the second tanh cancels out of alpha
                cf = st([P, c], "cf")
                V.tensor_scalar(out=cf[:], in0=argt[:], scalar1=1e5,
                                scalar2=0.1, op0=OP.mult, op1=OP.min)
                ryn = st([P, c], "ryn")
                V.reciprocal(ryn[:], yns[:])
                db = st([P, c], "db")
                V.tensor_scalar(out=db[:], in0=argt[:], scalar1=ATH_MAXN,
                                scalar2=2.0, op0=OP.min, op1=OP.mult)
                a1 = st([P, c], "a1")
                V.tensor_mul(a1[:], ryn[:], db[:])
                a2 = st([P, c], "a2")
                V.tensor_mul(a2[:], a1[:], cf[:])
                mask = st([P, c], "mask")
                V.tensor_scalar(out=mask[:], in0=qq[:, c:2 * c], scalar1=0.0,
                                scalar2=None, op0=OP.is_gt)
                alm = st([P, c], "alm")
                V.tensor_mul(alm[:], a2[:], mask[:])
                # scale in place (bf16 4x mode) + DMA out; factor 50 folds
                # the logmap 10/nrm and the artanh halves. Single-tile
                # waves scale+store in halves so the two DMAs overlap.
                for i, rt in enumerate(wave):
                    yt = y_sbs[rt]
                    nh = split_out
                    for h in range(nh):
                        sl = slice(h * (D // nh), (h + 1) * (D // nh))
                        V.tensor_scalar(out=yt[:, sl], in0=yt[:, sl],
                                        scalar1=alm[:, i:i + 1], scalar2=50.0,
                                        op0=OP.mult, op1=OP.mult)
                        nc.gpsimd.dma_start(
                            out=out_d[rt * P:(rt + 1) * P, sl],
                            in_=yt[:, sl])

            for wi, wave in enumerate(WAVES):
                cw = len(wave)
                # one shared PSUM tile for the wave's Gram accumulators:
                # per-rt column slices would be concurrent accumulation
                # groups in one 2KB zero region, so zero it explicitly and
                # accumulate with start=False throughout
                gram = psg.tile([P, cw * P], f32, tag="gram", name="gram")
                V.memset(gram[:], 0.0)
                qp = st([P, cw * 4], "qp")
                # early waves batch the qy squares big; late waves keep
                # them fine-grained so alpha (and the store) comes sooner
                sq_span = 4 if cw > 1 else 2
                for g in range(8):          # 512-col groups: blocks 2g, 2g+1
                    for i, rt in enumerate(wave):
                        py = psy.tile([P, 512], f32, tag=f"py{i}",
                                      name=f"py{i}", bufs=PY_BUFS[i])
                        for c in range(4):  # chunks 4g .. 4g+3
                            kc = 4 * g + c
                            lhs = xs(kc, rt)
                            nc.tensor.matmul(
                                py[:, (c // 2) * BS:(c // 2 + 1) * BS],
                                lhs, w_sb[:, kc * BS:(kc + 1) * BS],
                                start=(c % 2 == 0), stop=(c % 2 == 1),
                            )
                            nc.tensor.matmul(
                                gram[:, i * P:(i + 1) * P], lhs, lhs,
                                start=False, stop=False,
                                skip_group_check=True,
                            )
                        # drain PSUM -> bf16 y; ACT takes 2 of 8 groups,
                        # DVE the rest (ACT's budget goes to the squares)
                        y_sl = y_sbs[rt][:, g * 512:(g + 1) * 512]
                        if g in (2, 6):
                            nc.scalar.activation(y_sl, py[:], AF.Copy)
                        else:
                            V.tensor_copy(y_sl, py[:])
                        if g % sq_span == sq_span - 1:
                            # qy partial over the last sq_span groups
                            sq = sqpool.tile([P, 512 * sq_span], bf16,
                                             tag="sq", name="sq")
                            nc.scalar.activation(
                                sq[:],
                                y_sbs[rt][:, (g + 1 - sq_span) * 512:
                                          (g + 1) * 512],
                                AF.Square,
                                accum_out=qp[:, i * 4 + g // sq_span:
                                             i * 4 + g // sq_span + 1])
                # wave end: qx from gram diagonals, qy from qp sums
                nparts = 8 // sq_span
                qq = st([P, 2 * cw], "qq")
                for i, rt in enumerate(wave):
                    gsc = sqpool.tile([P, P], f32, tag="gsc", name="gsc")
                    V.tensor_mul(gsc[:], gram[:, i * P:(i + 1) * P], idm_sb[:])
                    V.reduce_sum(qq[:, i:i + 1], gsc[:],
                                 axis=mybir.AxisListType.X)
                    V.reduce_sum(qq[:, cw + i:cw + i + 1],
                                 qp[:, i * 4:i * 4 + nparts],
                                 axis=mybir.AxisListType.X)
                emit_chain(qq, cw, wave,
                           split_out=(2 if cw == 1 else 1))
    nc.finalize()
    return nc


_NC = None


def _get_nc():
    global _NC
    if _NC is None:
        _NC = build_nc()
    return _NC


def _prep_weights(weights: np.ndarray) -> np.ndarray:
    # w_sb[p, (2r+c)*256+j] = W[r, j, k=c*128+p]; bf16.
    wt = (weights.astype(np.float32).transpose(0, 2, 1)      # [r, k, j]
          .reshape(R, 2, P, BS).transpose(2, 0, 1, 3)        # [p, r, c, j]
          .reshape(P, WCOLS))
    return np.ascontiguousarray(wt).astype(ml_dtypes.bfloat16)


def _in_maps(x: np.ndarray, weights: np.ndarray) -> list:
    xf = np.ascontiguousarray(x, dtype=np.float32).reshape(
        N_CORES, ROWS_CORE, D)
    # host-side transpose to k-major + bf16 cast: [core, D, rows]
    xts = xf.transpose(0, 2, 1).astype(ml_dtypes.bfloat16)
    wid = _prep_weights(np.asarray(weights))
    idm = np.eye(P, dtype=np.float32)
    return [
        {"xt": np.ascontiguousarray(xts[i]), "w": wid, "idm": idm}
        for i in range(N_CORES)
    ]


def kernel(x: np.ndarray, weights: np.ndarray) -> np.ndarray:
    nc = _get_nc()
    in_maps = _in_maps(x, weights)
    res = run_bass_kernel_spmd(nc, in_maps, list(range(N_CORES)))
    out = np.concatenate([res.results[i]["out"] for i in range(N_CORES)],
                         axis=0)
    return out.reshape(x.shape).astype(np.float32)


if __name__ == "__main__":
    xs = np.random.randn(4, 2048, D).astype(np.float32)
    ws = (np.broadcast_to(np.eye(BS, dtype=np.float32), (R, BS, BS))
          + 0.02 * np.random.randn(R, BS, BS).astype(np.float32))
    o = kernel(xs, ws)
    print("kernel ran, out shape", o.shape, o.dtype)


# revision 26
# speedup vs baseline: 1.1517x; 1.0191x over previous
"""Trainium2 Bass kernel for nn_BlockDiagonalLinear_text (hyperbolic block-diag linear).

Math: the reference's per-row operations reduce to
  out = alpha_row * y   with  y = x @ blockdiag(W_1..W_16).T
where alpha_row is a chain of tanh/artanh scalars of ||x_row|| and
||y_row|| (the expmap0 scale cancels; validated numerically at 1.6e-4).

Sharding: data-parallel over rows; 8192 rows -> 8 cores x 1024 rows,
weights replicated (bf16).

Layout: x is transposed and cast to bf16 on the HOST, so the device
receives xT [D, rows] k-major — the exact stationary-operand layout the
PE needs (no on-chip transposes) at half the fp32 input bytes. Output
is written bf16 and upcast on the host, halving the output DMA.

Per-core schedule: 16 block-DMAs of xT stream on the scalar-engine
HWDGE ring while w streams on the sync ring. Row-tiles run in waves
[4,2,1,1]: wave 0 (4 tiles) is paced by the input stream, later waves
run from SBUF, and the two single-tile waves keep the final
drain+norm+scale+store tail short. Per (chunk, row-tile): y block
matmul (N=256) + Gram matmul (N=128, diag = ||x||^2) sharing the
stationary operand. PSUM y groups ([128,512] = 2 blocks) drain to bf16
via ACT (2 of 8 groups) and DVE (the rest); ACT Square+accum over
drained y gives ||y||^2; a per-wave Ln/Exp scalar chain produces
alpha; DVE scales y in place (bf16 4x mode) and the result streams out
over SWDGE.
"""
import sys
import numpy as np

for _p in ("/opt/trn_rl_repo", "/root/.axon_site/_ro/trn_rl_repo"):
    if _p not in sys.path:
        sys.path.append(_p)

import ml_dtypes
import concourse.bass as bass
import concourse.bacc as bacc
import concourse.mybir as mybir
from concourse import tile
from concourse.bass_utils import run_bass_kernel_spmd
from concourse.hw_specs import get_activation_tables

R, BS = 16, 256           # 16 diagonal blocks of 256x256
D = R * BS                # 4096
P = 128                   # partitions
N_CORES = 8
ROWS_TOTAL = 4 * 2048     # 8192
ROWS_CORE = ROWS_TOTAL // N_CORES   # 1024
NT = ROWS_CORE // P       # 8 row-tiles of 128 rows per core
NC = D // P               # 32 k-chunks of 128
WCOLS = 2 * R * BS        # 8192 weight columns

# wave 0 hides under the input stream; the trailing 1-tile waves keep
# the end-of-kernel drain/scale/store tail short
WAVES = [(0, 1, 2, 3), (4, 5), (6,), (7,)]
PY_BUFS = {0: 2, 1: 2, 2: 1, 3: 1}   # PSUM: 12KB y + 4KB gram per part

f32 = mybir.dt.float32
bf16 = mybir.dt.bfloat16
AF = mybir.ActivationFunctionType
OP = mybir.AluOpType

CLIP_Z = float(np.float32(1.0) - np.float32(1e-5))          # 0.99999
MAXNORM = float(np.float32(1.0 - 1e-3) / np.float32(0.1))   # 9.99
# artanh(min(tanh(t), c)) == min(t, artanh(c)) -- the clamps collapse to
# min-with-constant, removing both tanh+artanh evaluations from the chain
ATH_CLIPZ = float(np.arctanh(np.float64(CLIP_Z)))           # 6.1030
ATH_MAXN = float(np.arctanh(np.float64(np.float32(0.1) * np.float32(MAXNORM))))


def build_nc():
    nc = bacc.Bacc()
    xt_d = nc.declare_dram_parameter("xt", [D, ROWS_CORE], bf16, isOutput=False)
    w_d = nc.declare_dram_parameter("w", [P, WCOLS], bf16, isOutput=False)
    m_d = nc.declare_dram_parameter("idm", [P, P], f32, isOutput=False)
    out_d = nc.declare_dram_parameter("out", [ROWS_CORE, D], bf16, isOutput=True)

    tabs = list(get_activation_tables(nc.m.arch).items())
    nle_id = next(i for i, (n, _) in enumerate(tabs)
                  if n == "natural_log_exp_and_others")

    with tile.TileContext(nc) as tc:
        with (
            tc.tile_pool(name="wpool", bufs=1) as wpool,
            tc.tile_pool(name="xpool", bufs=1) as xpool,
            tc.tile_pool(name="ypool", bufs=1) as ypool,
            tc.tile_pool(name="sqpool", bufs=2) as sqpool,
            tc.tile_pool(name="stats", bufs=3) as stats,
            tc.tile_pool(name="psy", bufs=2, space="PSUM") as psy,
            tc.tile_pool(name="psg", bufs=2, space="PSUM") as psg,
        ):
            V = nc.vector

            # x streams on the sync-engine HWDGE ring; the small w + idm
            # loads go on the scalar ring, which drains before ACT's
            # first compute is needed — two descriptor rings in parallel
            xt_sb = xpool.tile([P, NC * ROWS_CORE], bf16, name="xt_sb")
            for b in range(R):
                src = xt_d[b * 2 * P:(b + 1) * 2 * P, :].rearrange(
                    "(c p) r -> p c r", p=P)
                nc.sync.dma_start(
                    out=xt_sb[:, b * 2 * ROWS_CORE:(b + 1) * 2 * ROWS_CORE],
                    in_=src)
            w_sb = wpool.tile([P, WCOLS], bf16, name="w_sb")
            for b in range(8):
                nc.scalar.dma_start(
                    out=w_sb[:, b * 1024:(b + 1) * 1024],
                    in_=w_d[:, b * 1024:(b + 1) * 1024])
            idm_sb = wpool.tile([P, P], f32, name="idm_sb")
            nc.scalar.dma_start(out=idm_sb[:], in_=m_d[:])

            # ACT: preload the one table set with ln+exp+copy so the
            # auto-inserted per-function loads all become no-ops.
            nc.scalar.add_instruction(mybir.InstLoadActFuncSet(
                name=nc.get_next_instruction_name(),
                act_func_set_id=nle_id, ins=[], outs=[]))

            def xs(kc, rt):
                # lhsT slice: [k=128, rows 128] of chunk kc, row-tile rt
                base = kc * ROWS_CORE + rt * P
                return xt_sb[:, base:base + P]

            def st(shape, tag):
                return stats.tile(shape, f32, tag=tag, name=tag)

            y_sbs = [ypool.tile([P, D], bf16, name=f"y_{rt}") for rt in range(NT)]


            def emit_chain(qq, c, wave, split_out):
                # qq: [P, 2c] = [qx cols | qy cols]
                lnq = st([P, 2 * c], "lnq")
                nc.scalar.activation(lnq[:], qq[:], AF.Ln)
                U = st([P, 2 * c], "U")   # [u | y_n] = sqrt via exp(.5 ln q)
                nc.scalar.activation(U[:], lnq[:], AF.Exp, scale=0.5)
                t1 = st([P, c], "t1")     # 0.1 * max(u, 1e-5)
                V.tensor_scalar(out=t1[:], in0=U[:, 0:c], scalar1=1e-5,
                                scalar2=0.1, op0=OP.max, op1=OP.mult)
                r1 = st([P, c], "r1")
                V.reciprocal(r1[:], t1[:])
                d_ = st([P, c], "d_")     # 2*artanh(min(tanh(t1), CLIP_Z))
                V.tensor_scalar(out=d_[:], in0=t1[:], scalar1=ATH_CLIPZ,
                                scalar2=2.0, op0=OP.min, op1=OP.mult)
                yns = st([P, c], "yns")
                V.tensor_scalar_max(yns[:], U[:, c:2 * c], 1e-20)
                w1 = st([P, c], "w1")
                V.tensor_mul(w1[:], U[:, c:2 * c], r1[:])
                w2 = st([P, c], "w2")
                V.tensor_mul(w2[:], w1[:], d_[:])
                argt = st([P, c], "argt")
                V.tensor_scalar(out=argt[:], in0=w2[:], scalar1=0.05,
                                scalar2=15.0, op0=OP.mult, op1=OP.min)
                # tanh(argt)/max(10*tanh(argt),1e-5) == min(1e5*argt, 0.1)
                # exactly in fp32, so the second tanh cancels out of alpha
                cf = st([P, c], "cf")
                V.tensor_scalar(out=cf[:], in0=argt[:], scalar1=1e5,
                                scalar2=0.1, op0=OP.mult, op1=OP.min)
                ryn = st([P, c], "ryn")
                V.reciprocal(ryn[:], yns[:])
                db = st([P, c], "db")
                V.tensor_scalar(out=db[:], in0=argt[:], scalar1=ATH_MAXN,
                                scalar2=2.0, op0=OP.min, op1=OP.mult)
                a1 = st([P, c], "a1")
                V.tensor_mul(a1[:], ryn[:], db[:])
                a2 = st([P, c], "a2")
                V.tensor_mul(a2[:], a1[:], cf[:])
                # qy == 0 already yields a2 == 0 through ln->exp (U=0,
                # w1=w2=argt=cf=db=0), so no explicit zero-mask is needed
                alm = a2
                # scale in place (bf16 4x mode) + DMA out; factor 50 folds
                # the logmap 10/nrm and the artanh halves. Single-tile
                # waves scale+store in halves so the two DMAs overlap.
                for i, rt in enumerate(wave):
                    yt = y_sbs[rt]
                    nh = split_out
                    for h in range(nh):
                        sl = slice(h * (D // nh), (h + 1) * (D // nh))
                        V.tensor_scalar(out=yt[:, sl], in0=yt[:, sl],
                                        scalar1=alm[:, i:i + 1], scalar2=50.0,
                                        op0=OP.mult, op1=OP.mult)
                        nc.gpsimd.dma_start(
                            out=out_d[rt * P:(rt + 1) * P, sl],
                            in_=yt[:, sl])

            for wi, wave in enumerate(WAVES):
                cw = len(wave)
                # one shared PSUM tile for the wave's Gram accumulators:
                # per-rt column slices would be concurrent accumulation
                # groups in one 2KB zero region, so zero it explicitly and
                # accumulate with start=False throughout
                gram = psg.tile([P, cw * P], f32, tag="gram", name="gram")
                V.memset(gram[:], 0.0)
                qp = st([P, cw * 4], "qp")
                gw = 512                          # group width (cols)
                ng = D // gw                      # groups per rt
                act_g = (2, 6)
                sq_cols = 2048 if cw > 1 else 1024
                sq_every = sq_cols // gw
                for g in range(ng):
                    for i, rt in enumerate(wave):
                        py = psy.tile([P, gw], f32, tag=f"py{i}",
                                      name=f"py{i}", bufs=PY_BUFS[i])
                        for c in range(gw // P):  # chunks per group
                            kc = (gw // P) * g + c
                            lhs = xs(kc, rt)
                            nc.tensor.matmul(
                                py[:, (c // 2) * BS:(c // 2 + 1) * BS],
                                lhs, w_sb[:, kc * BS:(kc + 1) * BS],
                                start=(c % 2 == 0), stop=(c % 2 == 1),
                            )
                            nc.tensor.matmul(
                                gram[:, i * P:(i + 1) * P], lhs, lhs,
                                start=False, stop=False,
                                skip_group_check=True,
                            )
                        # drain PSUM -> bf16 y; ACT takes ~1/4 of the
                        # groups, DVE the rest (ACT's budget is the squares)
                        y_sl = y_sbs[rt][:, g * gw:(g + 1) * gw]
                        if g in act_g:
                            nc.scalar.activation(y_sl, py[:], AF.Copy)
                        else:
                            V.tensor_copy(y_sl, py[:])
                        if g % sq_every == sq_every - 1:
                            # qy partial over the last sq_cols columns
                            sq = sqpool.tile([P, sq_cols], bf16,
                                             tag="sq", name="sq")
                            nc.scalar.activation(
                                sq[:],
                                y_sbs[rt][:, (g + 1) * gw - sq_cols:
                                          (g + 1) * gw],
                                AF.Square,
                                accum_out=qp[:, i * 4 + g // sq_every:
                                             i * 4 + g // sq_every + 1])
                # wave end: qx from gram diagonals, qy from qp sums
                nparts = ng // sq_every
                qq = st([P, 2 * cw], "qq")
                for i, rt in enumerate(wave):
                    gsc = sqpool.tile([P, P], f32, tag="gsc", name="gsc")
                    V.tensor_mul(gsc[:], gram[:, i * P:(i + 1) * P], idm_sb[:])
                    V.reduce_sum(qq[:, i:i + 1], gsc[:],
                                 axis=mybir.AxisListType.X)
                    V.reduce_sum(qq[:, cw + i:cw + i + 1],
                                 qp[:, i * 4:i * 4 + nparts],
                                 axis=mybir.AxisListType.X)
                emit_chain(qq, cw, wave,
                           split_out=(2 if cw == 1 else 1))
    nc.finalize()
    return nc


_NC = None


def _get_nc():
    global _NC
    if _NC is None:
        _NC = build_nc()
    return _NC


def _prep_weights(weights: np.ndarray) -> np.ndarray:
    # w_sb[p, (2r+c)*256+j] = W[r, j, k=c*128+p]; bf16.
    wt = (weights.astype(np.float32).transpose(0, 2, 1)      # [r, k, j]
          .reshape(R, 2, P, BS).transpose(2, 0, 1, 3)        # [p, r, c, j]
          .reshape(P, WCOLS))
    return np.ascontiguousarray(wt).astype(ml_dtypes.bfloat16)


def _in_maps(x: np.ndarray, weights: np.ndarray) -> list:
    xf = np.ascontiguousarray(x, dtype=np.float32).reshape(
        N_CORES, ROWS_CORE, D)
    # host-side transpose to k-major + bf16 cast: [core, D, rows]
    xts = xf.transpose(0, 2, 1).astype(ml_dtypes.bfloat16)
    wid = _prep_weights(np.asarray(weights))
    idm = np.eye(P, dtype=np.float32)
    return [
        {"xt": np.ascontiguousarray(xts[i]), "w": wid, "idm": idm}
        for i in range(N_CORES)
    ]


def kernel(x: np.ndarray, weights: np.ndarray) -> np.ndarray:
    nc = _get_nc()
    in_maps = _in_maps(x, weights)
    res = run_bass_kernel_spmd(nc, in_maps, list(range(N_CORES)))
    out = np.concatenate([res.results[i]["out"] for i in range(N_CORES)],
                         axis=0)
    return out.reshape(x.shape).astype(np.float32)


if __name__ == "__main__":
    xs = np.random.randn(4, 2048, D).astype(np.float32)
    ws = (np.broadcast_to(np.eye(BS, dtype=np.float32), (R, BS, BS))
          + 0.02 * np.random.randn(R, BS, BS).astype(np.float32))
    o = kernel(xs, ws)
    print("kernel ran, out shape", o.shape, o.dtype)
